# revision 1
# baseline (speedup 1.0000x reference)
"""Trainium2 Bass kernel for DissimilarityMixtureEncoderCov forward.

Computes softmax(-ALPHA * D + log(relu(mixers)), axis=-1) where
  D[b,k] = (x_b - mu_k)^T (C_k C_k^T) (x_b - mu_k)

Data-parallel over batch across 8 NeuronCores. Per core, using the identity
D = ||C^T x - C^T mu||^2 expanded in three terms:

  Y[b,(k,j)] = x_b . C_k[:,j]   -- split into 3 fast matmuls accumulated in
      PSUM:  fp32r(Xr)·fp32r(Cr) + bf16(Ex)·bf16(Cb) + bf16(Xb)·bf16(Ec)
      where Xr/Cr are fp32r-rounded x/cov, Ex/Ec the rounding residuals
      (bf16), Xb/Cb bf16 copies. Full-precision products + fp32 PSUM
      accumulation keep the result at fp32-envelope accuracy at 3 PE
      cycles/row instead of native fp32's 4.
  T1[b,k]  = ALPHA * sum_j Y^2  -- ACT square + DVE grouped reduce for NDVE
      of 16 k-groups per block; fused ACT square+accum for the rest.
  t_k = C_k^T mu_k, v_k = C_k t_k  -- DVE broadcast-mult + reduce.
  logits = -T1 + 2a*x.v + (-a*||t||^2 + log(mixers)); softmax fused.

Loop nest: cov column-blocks outer (streamed from HBM through small rings,
converted per block), batch chunks inner.
"""

import sys

sys.path.insert(0, "/opt/trn_rl_repo")

import numpy as np

import concourse.bacc as bacc
import concourse.tile as tile
from concourse import mybir

ALPHA = 10.0
B, K, D = 8192, 128, 128
N_CORES = 8
B_LOC = B // N_CORES          # 1024 batch rows per core
N_CHUNKS = B_LOC // 128       # 8 chunks of 128 rows
KJ = K * D                    # 16384 columns of the big matmul
BLK = 2048                    # psum block = 4 banks; 16 k-groups
N_BLK = KJ // BLK             # 8 blocks
NGRP = BLK // 128             # 16 k-groups per block
NDVE = 15                     # k-groups reduced on DVE per block
NFUSE = NGRP - NDVE           # k-groups via fused ACT square+accum
SQRT_A = float(np.sqrt(ALPHA))

FP32 = mybir.dt.float32
FP32R = mybir.dt.float32r
BF16 = mybir.dt.bfloat16


def _build_bass():
    nc = bacc.Bacc("TRN2", target_bir_lowering=False, debug=False,
                   num_devices=N_CORES)

    x_d = nc.dram_tensor("x", [B_LOC, D], FP32, kind="ExternalInput")
    cov_d = nc.dram_tensor("cov", [K * D, D], FP32, kind="ExternalInput")
    cen_d = nc.dram_tensor("centers", [K, D], FP32, kind="ExternalInput")
    mix_d = nc.dram_tensor("mixers", [1, K], FP32, kind="ExternalInput")
    ident_d = nc.dram_tensor("ident", [128, 128], FP32, kind="ExternalInput")
    out_d = nc.dram_tensor("out", [B_LOC, K], FP32, kind="ExternalOutput")

    AF = mybir.ActivationFunctionType
    OP = mybir.AluOpType
    AX = mybir.AxisListType

    with tile.TileContext(nc) as tc:
        with (
            tc.tile_pool(name="const", bufs=1) as constp,
            tc.tile_pool(name="covf", bufs=2) as covfp,    # fp32 cov blocks
            tc.tile_pool(name="chr", bufs=2) as chrp,      # fp32r cov blocks
            tc.tile_pool(name="cbb", bufs=2) as cbbp,      # bf16 cov blocks
            tc.tile_pool(name="ecb", bufs=2) as ecbp,      # bf16 residuals
            tc.tile_pool(name="covk", bufs=1) as covkp,
            tc.tile_pool(name="prod", bufs=2) as prodp,
            tc.tile_pool(name="xt", bufs=1) as xtp,
            tc.tile_pool(name="small", bufs=1) as smallp,
            tc.tile_pool(name="work", bufs=4) as workp,
            tc.tile_pool(name="t1a", bufs=1) as t1ap,
            tc.tile_pool(name="ysq", bufs=4) as ysqp,
            tc.tile_pool(name="py", bufs=2, space="PSUM") as pyp,
        ):
            # ---------- small inputs ----------
            x_sb = smallp.tile([128, N_CHUNKS * 128], FP32)  # [b, (c,d)]
            nc.sync.dma_start(
                out=x_sb[:, :].rearrange("b (c d) -> b c d", d=128),
                in_=x_d[:, :].rearrange("(c b) d -> b c d", b=128),
            )
            ident = constp.tile([128, 128], FP32)
            nc.sync.dma_start(out=ident[:, :], in_=ident_d[:, :])
            cen_sb = smallp.tile([128, 128], FP32)   # [k, d]
            nc.sync.dma_start(out=cen_sb[:, :], in_=cen_d[:, :])
            mix = smallp.tile([1, K], FP32)
            nc.sync.dma_start(out=mix[:, :], in_=mix_d[:, :])
            covk_sb = covkp.tile([128, KJ], FP32)

            def prep_block(blk):
                c0 = blk * BLK
                covf = covfp.tile([128, BLK], FP32, tag="covf")
                nc.sync.dma_start(
                    out=covf[:, :].rearrange("d (g j) -> d g j", j=128),
                    in_=cov_d[c0:c0 + BLK, :].rearrange(
                        "(g d) j -> d g j", d=128),
                )
                chrt = chrp.tile([128, BLK], FP32R, tag="chr")
                nc.vector.tensor_copy(chrt[:, :], covf[:, :])
                ecbt = ecbp.tile([128, BLK], BF16, tag="ecb")
                nc.gpsimd.tensor_tensor(out=ecbt[:, :], in0=covf[:, :],
                                        in1=chrt[:, :].bitcast(FP32),
                                        op=OP.subtract)
                cbbt = cbbp.tile([128, BLK], BF16, tag="cbb")
                nc.scalar.copy(cbbt[:, :], covf[:, :])
                return chrt, ecbt, cbbt

            prepped = prep_block(0)

            # ---------- transpose x; build split variants per chunk ----
            xt_sb = xtp.tile([128, B_LOC], FP32)            # [d, b]
            xtr = xtp.tile([128, B_LOC], FP32R)             # rounded
            exb = xtp.tile([128, B_LOC], BF16)              # residual
            xbb = xtp.tile([128, B_LOC], BF16)              # bf16 of rounded
            for c in range(N_CHUNKS):
                sl = slice(c * 128, (c + 1) * 128)
                tp = pyp.tile([128, 128], FP32, tag="py")
                nc.tensor.transpose(tp[:, :], x_sb[:, sl], ident[:, :])
                nc.scalar.copy(xt_sb[:, sl], tp[:, :])
                nc.vector.tensor_copy(xtr[:, sl], xt_sb[:, sl])
                nc.vector.tensor_tensor(out=exb[:, sl], in0=xt_sb[:, sl],
                                        in1=xtr[:, sl].bitcast(FP32),
                                        op=OP.subtract)
                nc.vector.tensor_copy(xbb[:, sl], xtr[:, sl].bitcast(FP32))

            # small helpers
            ones_row = constp.tile([1, 128], FP32)
            nc.vector.memset(ones_row[:, :], 1.0)
            ones_col = constp.tile([128, 1], FP32)
            nc.vector.memset(ones_col[:, :], 1.0)
            bias_row = smallp.tile([1, K], FP32)
            nc.vector.tensor_scalar_max(bias_row[:, :], mix[:, :], 0.0)
            nc.scalar.activation(bias_row[:, :], bias_row[:, :], AF.Ln)

            t_sb = smallp.tile([128, 128], FP32)     # [k, j]
            v_sb = smallp.tile([128, 128], FP32)     # [k, d]
            JG = 16
            DG = 16
            cen_bc = cen_sb[:, :].rearrange(
                "k (d o) -> k d o", o=1).broadcast_to([128, 128, JG])
            t_bc = t_sb[:, :].rearrange(
                "k (o j) -> k o j", o=1).broadcast_to([128, DG, 128])

            def t_slice(i):
                j0 = i * JG
                prod = prodp.tile([128, 128 * JG], FP32, tag="prod")
                nc.gpsimd.tensor_tensor(
                    out=prod[:, :].rearrange("k (d j) -> k d j", j=JG),
                    in0=covk_sb[:, :].rearrange(
                        "k (d j) -> k d j", j=128)[:, :, j0:j0 + JG],
                    in1=cen_bc, op=OP.mult)
                nc.vector.tensor_reduce(
                    out=t_sb[:, j0:j0 + JG],
                    in_=prod[:, :].rearrange("k (d j) -> k j d", j=JG),
                    axis=AX.X, op=OP.add)

            def v_slice(i):
                d0 = i * DG
                prod = prodp.tile([128, DG * 128], FP32, tag="prod")
                nc.gpsimd.tensor_tensor(
                    out=prod[:, :].rearrange("k (d j) -> k d j", j=128),
                    in0=covk_sb[:, d0 * 128:(d0 + DG) * 128].rearrange(
                        "k (d j) -> k d j", j=128),
                    in1=t_bc, op=OP.mult)
                nc.vector.tensor_reduce(
                    out=v_sb[:, d0:d0 + DG],
                    in_=prod[:, :].rearrange("k (d j) -> k d j", j=128),
                    axis=AX.X, op=OP.add)

            # ---------- phase 1: blocks outer, chunks inner ----------
            t1a_all = []
            for c in range(N_CHUNKS):
                t1a_c = t1ap.tile([128, K], FP32, tag=f"t1a{c}")
                t1a_all.append(t1a_c)

            const_row = smallp.tile([1, K], FP32)

            def const_chain():
                # const row: -ALPHA*||t_k||^2 + bias_k  (t complete at blk 5)
                tsq = smallp.tile([128, 128], FP32)
                nc.scalar.activation(tsq[:, :], t_sb[:, :], AF.Square)
                tsqt_p = pyp.tile([128, 128], FP32, tag="py")
                nc.tensor.transpose(tsqt_p[:, :], tsq[:, :], ident[:, :])
                tsqt = smallp.tile([128, 128], FP32)     # [j, k]
                nc.scalar.copy(tsqt[:, :], tsqt_p[:, :])
                crow_p = pyp.tile([1, 128], FP32, tag="py")
                nc.tensor.matmul(crow_p[:, :], ones_col[:, :], tsqt[:, :],
                                 start=True, stop=True)
                nc.scalar.activation(const_row[:, :], crow_p[:, :], AF.Copy,
                                     scale=-ALPHA)
                nc.vector.tensor_tensor(out=const_row[:, :],
                                        in0=const_row[:, :],
                                        in1=bias_row[:, :], op=OP.add)

            covk_loaded = False
            for blk in range(N_BLK):
                if blk == 6:
                    const_chain()
                chrt, ecbt, cbbt = prepped
                if blk + 1 < N_BLK:
                    prepped = prep_block(blk + 1)
                if not covk_loaded:
                    # covk DMA queued after the first two cov blocks
                    nc.sync.dma_start(
                        out=covk_sb[:, :].rearrange("k (d j) -> k d j", j=128),
                        in_=cov_d[:, :].rearrange("(k d) j -> k d j", d=128),
                    )
                    covk_loaded = True

                for c in range(N_CHUNKS):
                    py = pyp.tile([128, BLK], FP32, tag="py")
                    # main term: fp32r x fp32r, 512-col regions
                    for m in range(BLK // 512):
                        nc.tensor.matmul(
                            py[:, m * 512:(m + 1) * 512],
                            xtr[:, c * 128:(c + 1) * 128],
                            chrt[:, m * 512:(m + 1) * 512],
                            start=True, stop=False, skip_group_check=True)
                    # corrections: bf16, 512-col regions (psum bank limit)
                    for m in range(BLK // 512):
                        nc.tensor.matmul(
                            py[:, m * 512:(m + 1) * 512],
                            exb[:, c * 128:(c + 1) * 128],
                            cbbt[:, m * 512:(m + 1) * 512],
                            start=False, stop=False, skip_group_check=True)
                    for m in range(BLK // 512):
                        nc.tensor.matmul(
                            py[:, m * 512:(m + 1) * 512],
                            xbb[:, c * 128:(c + 1) * 128],
                            ecbt[:, m * 512:(m + 1) * 512],
                            start=False, stop=True, skip_group_check=True)

                    t1a = t1a_all[c]
                    # first NDVE k-groups: bulk ACT square -> DVE reduce
                    w = NDVE * 128
                    ysq = ysqp.tile([128, w], FP32, tag="ysq")
                    nc.scalar.activation(ysq[:, :], py[:, 0:w], AF.Square,
                                         scale=SQRT_A)
                    nc.vector.tensor_reduce(
                        out=t1a[:, blk * NGRP:blk * NGRP + NDVE],
                        in_=ysq[:, :].rearrange("b (g j) -> b g j", j=128),
                        axis=AX.X, op=OP.add)
                    # last NFUSE k-groups: fused ACT square+accum from PSUM
                    for f in range(NFUSE):
                        g = NDVE + f
                        sc = workp.tile([128, 128], FP32, tag="sqscratch")
                        nc.scalar.activation(
                            sc[:, :], py[:, g * 128:(g + 1) * 128],
                            AF.Square, scale=SQRT_A,
                            accum_out=t1a[:, blk * NGRP + g:
                                          blk * NGRP + g + 1])
                    # t/v slices: three per block starting at block 2
                    if blk >= 2 and c in (2, 4, 6):
                        i = (blk - 2) * 3 + (2, 4, 6).index(c)
                        if i < 8:
                            t_slice(i)
                        elif i < 16:
                            v_slice(i - 8)

            # vt2a[d, k] = 2*ALPHA * v[k, d]^T
            vt2a_sb = smallp.tile([128, 128], FP32)
            tpv = pyp.tile([128, 128], FP32, tag="py")
            nc.tensor.transpose(tpv[:, :], v_sb[:, :], ident[:, :])
            nc.scalar.activation(vt2a_sb[:, :], tpv[:, :], AF.Copy,
                                 scale=2.0 * ALPHA)


            # ---------- phase 2: logits + softmax ----------
            for c in range(N_CHUNKS):
                lhsT = xt_sb[:, c * 128:(c + 1) * 128]
                t1a = t1a_all[c]

                pl = pyp.tile([128, K], FP32, tag="py")
                nc.tensor.matmul(pl[:, :], lhsT, vt2a_sb[:, :],
                                 start=True, stop=False)
                nc.tensor.matmul(pl[:, :], ones_row[:, :], const_row[:, :],
                                 start=False, stop=True)

                lg = workp.tile([128, K], FP32, tag="lg")
                nc.vector.tensor_tensor(out=lg[:, :], in0=pl[:, :],
                                        in1=t1a[:, :], op=OP.subtract)
                mx = workp.tile([128, 1], FP32, tag="mx")
                nc.vector.tensor_reduce(out=mx[:, :], in_=lg[:, :],
                                        axis=AX.X, op=OP.max)
                nmx = workp.tile([128, 1], FP32, tag="nmx")
                nc.vector.tensor_scalar_mul(nmx[:, :], mx[:, :], -1.0)
                ex = workp.tile([128, K], FP32, tag="ex")
                den = workp.tile([128, 1], FP32, tag="den")
                nc.scalar.activation(ex[:, :], lg[:, :], AF.Exp,
                                     bias=nmx[:, 0:1], accum_out=den[:, 0:1])
                rden = workp.tile([128, 1], FP32, tag="rden")
                nc.vector.reciprocal(rden[:, :], den[:, :])
                ot = workp.tile([128, K], FP32, tag="ot")
                nc.vector.tensor_scalar(out=ot[:, :], in0=ex[:, :],
                                        scalar1=rden[:, 0:1], scalar2=None,
                                        op0=OP.mult)
                nc.sync.dma_start(out=out_d[c * 128:(c + 1) * 128, :],
                                  in_=ot[:, :])

    nc.compile()
    return nc


_NC_CACHE = None


def kernel(x, centers, cov, mixers):
    global _NC_CACHE
    from concourse.bass_utils import run_bass_kernel_spmd

    if _NC_CACHE is None:
        _NC_CACHE = _build_bass()
    nc = _NC_CACHE

    x = np.ascontiguousarray(x, dtype=np.float32)
    cov2 = np.ascontiguousarray(cov, dtype=np.float32).reshape(K * D, D)
    cen = np.ascontiguousarray(centers, dtype=np.float32)
    mix = np.ascontiguousarray(mixers, dtype=np.float32)
    ident = np.eye(128, dtype=np.float32)

    in_maps = []
    for c in range(N_CORES):
        in_maps.append({
            "x": x[c * B_LOC:(c + 1) * B_LOC],
            "cov": cov2,
            "centers": cen,
            "mixers": mix,
            "ident": ident,
        })
    res = run_bass_kernel_spmd(nc, in_maps, list(range(N_CORES)))
    out = np.concatenate([res.results[c]["out"] for c in range(N_CORES)],
                         axis=0)
    return out



# revision 2
# speedup vs baseline: 1.0124x; 1.0124x over previous
"""Trainium2 Bass kernel for DissimilarityMixtureEncoderCov forward.

Computes softmax(-ALPHA * D + log(relu(mixers)), axis=-1) where
  D[b,k] = (x_b - mu_k)^T (C_k C_k^T) (x_b - mu_k)

Data-parallel over batch across 8 NeuronCores. Per core, using the identity
D = ||C^T x - C^T mu||^2 expanded in three terms:

  Y[b,(k,j)] = x_b . C_k[:,j]   -- split into 3 fast matmuls accumulated in
      PSUM:  fp32r(Xr)·fp32r(Cr) + bf16(Ex)·bf16(Cb) + bf16(Xb)·bf16(Ec)
      (fp32-envelope accuracy at 3 PE cycles/row vs native fp32's 4).
      This is the engine floor: 3 passes x 16384 cols x 8 chunks ~ 164us.
  T1[b,k]  = ALPHA * sum_j Y^2  -- ACT squares each 1024-col PSUM tile;
      the grouped j-reduce is split DVE (tensor_reduce) / Pool (in-place
      binary-tree tensor_tensor adds; gpsimd tensor_reduce cannot do
      free-axis grouped reduces) by a static balance schedule.
  t_k = C_k^T mu_k  -- 16 free-size-1 fp32 PE matmuls per cov block
      (lhsT = C_k block, rhs = cenT column), accumulated into tT[j,k].
  v_k = C_k t_k -- per block: transpose tT slice to t_kj rows, flatten via
      a tiny SBUF->SBUF DMA, replicate across partitions with gpsimd
      partition_broadcast, then DVE mult + grouped reduce in covf layout
      -> vT[d,k] slice. Fully pipelined block-by-block; no [k,(d,j)]
      copy of cov needed at all.
  logits = -T1 + 2a*x.v + (-a*||t||^2 + log(mixers)); softmax fused.

All t/v stages are emitted at chunk boundaries inside the block loop so
every engine's FIFO queue sees instructions in ready-order (no
head-of-line blocking), keeping the PE gapless (p-state stays at max).
PSUM: 3 x 1024-col matmul tiles (6 banks) + small pool (2 banks).
"""

import sys

sys.path.insert(0, "/opt/trn_rl_repo")

import numpy as np

import concourse.bacc as bacc
import concourse.tile as tile
from concourse import mybir

ALPHA = 10.0
B, K, D = 8192, 128, 128
N_CORES = 8
B_LOC = B // N_CORES          # 1024 batch rows per core
N_CHUNKS = B_LOC // 128       # 8 chunks of 128 rows
KJ = K * D                    # 16384 columns of the big matmul
BLK = 2048                    # cov block; 16 k-groups
N_BLK = KJ // BLK             # 8 blocks
NGRP = BLK // 128             # 16 k-groups per block
HB = 1024                     # psum half-block (2 banks)
HGRP = HB // 128              # 8 k-groups per half
SQRT_A = float(np.sqrt(ALPHA))

# Of the 128 (block, chunk, half) T1 tiles, this many reduce on DVE
# (~1.13us/tile), the rest tree-reduce on Pool (~2.2us/tile).
N_RED_DVE = 72

FP32 = mybir.dt.float32
FP32R = mybir.dt.float32r
BF16 = mybir.dt.bfloat16


def _build_bass():
    nc = bacc.Bacc("TRN2", target_bir_lowering=False, debug=False,
                   num_devices=N_CORES)

    x_d = nc.dram_tensor("x", [B_LOC, D], FP32, kind="ExternalInput")
    cov_d = nc.dram_tensor("cov", [K * D, D], FP32, kind="ExternalInput")
    cen_d = nc.dram_tensor("centers", [K, D], FP32, kind="ExternalInput")
    mix_d = nc.dram_tensor("mixers", [1, K], FP32, kind="ExternalInput")
    ident_d = nc.dram_tensor("ident", [128, 128], FP32, kind="ExternalInput")
    out_d = nc.dram_tensor("out", [B_LOC, K], FP32, kind="ExternalOutput")

    AF = mybir.ActivationFunctionType
    OP = mybir.AluOpType
    AX = mybir.AxisListType

    with tile.TileContext(nc) as tc:
        with (
            tc.tile_pool(name="const", bufs=1) as constp,
            tc.tile_pool(name="covf", bufs=2) as covfp,    # fp32 cov blocks
            tc.tile_pool(name="chr", bufs=2) as chrp,      # fp32r cov blocks
            tc.tile_pool(name="cbb", bufs=2) as cbbp,      # bf16 cov blocks
            tc.tile_pool(name="ecb", bufs=2) as ecbp,      # bf16 residuals
            tc.tile_pool(name="prod", bufs=2) as prodp,    # v-mult scratch
            tc.tile_pool(name="tbc", bufs=2) as tbcp,      # t bcast tiles
            tc.tile_pool(name="tfl", bufs=2) as tflp,
            tc.tile_pool(name="tkj", bufs=2) as tkjp,      # t flat rows
            tc.tile_pool(name="xt", bufs=1) as xtp,
            tc.tile_pool(name="small", bufs=1) as smallp,
            tc.tile_pool(name="work", bufs=4) as workp,
            tc.tile_pool(name="t1a", bufs=1) as t1ap,
            tc.tile_pool(name="ysq", bufs=5) as ysqp,
            tc.tile_pool(name="py", bufs=3, space="PSUM") as pyp,
            tc.tile_pool(name="ps", bufs=2, space="PSUM") as psp,
        ):
            # ---------- small inputs ----------
            # small DMAs first (x transposes gate the first matmuls),
            # covf(0) right behind
            ident = constp.tile([128, 128], FP32)
            nc.sync.dma_start(out=ident[:, :], in_=ident_d[:, :])
            x_sb = smallp.tile([128, N_CHUNKS * 128], FP32)  # [b, (c,d)]
            nc.sync.dma_start(
                out=x_sb[:, :].rearrange("b (c d) -> b c d", d=128),
                in_=x_d[:, :].rearrange("(c b) d -> b c d", b=128),
            )
            cen_sb = smallp.tile([128, 128], FP32)   # [k, d]
            nc.sync.dma_start(out=cen_sb[:, :], in_=cen_d[:, :])
            mix = smallp.tile([1, K], FP32)
            nc.sync.dma_start(out=mix[:, :], in_=mix_d[:, :])
            covf0 = covfp.tile([128, BLK], FP32, tag="covf")
            nc.sync.dma_start(
                out=covf0[:, :].rearrange("d (g j) -> d g j", j=128),
                in_=cov_d[0:BLK, :].rearrange("(g d) j -> d g j", d=128),
            )

            def conv_block(covf, blk):
                chrt = chrp.tile([128, BLK], FP32R, tag="chr")
                nc.gpsimd.tensor_copy(chrt[:, :], covf[:, :])
                ecbt = ecbp.tile([128, BLK], BF16, tag="ecb")
                nc.gpsimd.tensor_tensor(out=ecbt[:, :], in0=covf[:, :],
                                        in1=chrt[:, :].bitcast(FP32),
                                        op=OP.subtract)
                cbbt = cbbp.tile([128, BLK], BF16, tag="cbb")
                nc.scalar.copy(cbbt[:, :], covf[:, :])
                return chrt, ecbt, cbbt

            def load_block(blk):
                c0 = blk * BLK
                covf = covfp.tile([128, BLK], FP32, tag="covf")
                nc.sync.dma_start(
                    out=covf[:, :].rearrange("d (g j) -> d g j", j=128),
                    in_=cov_d[c0:c0 + BLK, :].rearrange(
                        "(g d) j -> d g j", d=128),
                )
                return covf

            chrt0, ecbt0, cbbt0 = conv_block(covf0, 0)

            # ---------- x variants (emitted per chunk inside block 0) ----
            xt_sb = xtp.tile([128, B_LOC], FP32)            # [d, b]
            xtr = xtp.tile([128, B_LOC], FP32R)             # rounded
            exb = xtp.tile([128, B_LOC], BF16)              # residual
            xbb = xtp.tile([128, B_LOC], BF16)              # bf16 of rounded

            def x_prep(c):
                sl = slice(c * 128, (c + 1) * 128)
                tp = psp.tile([128, 128], FP32, tag="ps")
                nc.tensor.transpose(tp[:, :], x_sb[:, sl], ident[:, :])
                nc.scalar.copy(xt_sb[:, sl], tp[:, :])
                nc.vector.tensor_copy(xtr[:, sl], xt_sb[:, sl])
                nc.vector.tensor_tensor(out=exb[:, sl], in0=xt_sb[:, sl],
                                        in1=xtr[:, sl].bitcast(FP32),
                                        op=OP.subtract)
                nc.vector.tensor_copy(xbb[:, sl], xtr[:, sl].bitcast(FP32))

            for _c in range(N_CHUNKS):
                x_prep(_c)

            # cenT [d, k] via PE transpose
            cenT = smallp.tile([128, 128], FP32)
            cenT_p = psp.tile([128, 128], FP32, tag="ps")
            nc.tensor.transpose(cenT_p[:, :], cen_sb[:, :], ident[:, :])
            # fold 2*ALPHA into t (and thus v): phase 2 then uses vT as-is
            nc.scalar.activation(cenT[:, :], cenT_p[:, :], AF.Copy,
                                 scale=2.0 * ALPHA)

            # small helpers
            ones_row = constp.tile([1, 128], FP32)
            nc.vector.memset(ones_row[:, :], 1.0)
            ones_col = constp.tile([128, 1], FP32)
            nc.vector.memset(ones_col[:, :], 1.0)
            bias_row = smallp.tile([1, K], FP32)
            nc.vector.tensor_scalar_max(bias_row[:, :], mix[:, :], 0.0)
            nc.scalar.activation(bias_row[:, :], bias_row[:, :], AF.Ln)

            # t / v state
            tT_sb = smallp.tile([128, 128], FP32)    # [j, k]
            nc.vector.memset(tT_sb[:, :], 0.0)
            vT_sb = smallp.tile([128, 128], FP32)    # [d, k]

            t1a_all = []
            for c in range(N_CHUNKS):
                t1a_c = t1ap.tile([128, K], FP32, tag=f"t1a{c}")
                t1a_all.append(t1a_c)

            const_row = smallp.tile([1, K], FP32)

            def const_chain():
                # const row: -ALPHA*||t_k||^2 + bias_k
                tsqT = smallp.tile([128, 128], FP32)     # [j, k]
                nc.scalar.activation(tsqT[:, :], tT_sb[:, :], AF.Square)
                crow_p = psp.tile([1, 128], FP32, tag="ps")
                nc.tensor.matmul(crow_p[:, :], ones_col[:, :], tsqT[:, :],
                                 start=True, stop=True)
                nc.scalar.activation(const_row[:, :], crow_p[:, :], AF.Copy,
                                     scale=-1.0 / (4.0 * ALPHA))
                nc.vector.tensor_tensor(out=const_row[:, :],
                                        in0=const_row[:, :],
                                        in1=bias_row[:, :], op=OP.add)

            # ---- t/v pipeline stages for block blk (uses its covf tile)
            def t_stage1(blk, covf):
                # 16 tiny fp32 matmuls: tT[:, k] = C_k^T mu_k
                ksl = slice(blk * NGRP, (blk + 1) * NGRP)
                tps = psp.tile([128, NGRP], FP32, tag="ps")
                for g in range(NGRP):
                    k = blk * NGRP + g
                    nc.tensor.matmul(tps[:, g:g + 1],
                                     covf[:, g * 128:(g + 1) * 128],
                                     cenT[:, k:k + 1],
                                     start=True, stop=True)
                nc.vector.tensor_copy(tT_sb[:, ksl], tps[:, :])

            def t_stage2(blk):
                # transpose full tT; full PSUM->SBUF copy (the BIR verifier
                # rejects partition-offset PSUM reads)
                trp = psp.tile([128, 128], FP32, tag="ps")
                nc.tensor.transpose(trp[:, :], tT_sb[:, :], ident[:, :])
                tkjs = tkjp.tile([128, 128], FP32, tag="tkj")  # [k, j]
                nc.vector.tensor_copy(tkjs[:, :], trp[:, :])
                return tkjs

            def t_stage3(blk, tkjs):
                # flatten rows to [1, 2048], then broadcast to 128 parts
                ksl = slice(blk * NGRP, (blk + 1) * NGRP)
                tfl = tflp.tile([1, BLK], FP32, tag="tfl")
                nc.sync.dma_start(
                    out=tfl[0:1, :].rearrange("o (k j) -> o k j", j=128),
                    in_=tkjs[ksl, :].rearrange("k (o j) -> k o j", o=1),
                )
                tbct = tbcp.tile([128, BLK], FP32, tag="tbc")
                nc.gpsimd.partition_broadcast(tbct[:, :], tfl[0:1, :])
                return tbct

            def t_stage4(blk, covf, tbct):
                # v slice: vT[d, ksl] = sum_j covf[d,(k,j)] * t[k,j]
                ksl = slice(blk * NGRP, (blk + 1) * NGRP)
                prod = prodp.tile([128, BLK], FP32, tag="prod")
                nc.gpsimd.tensor_tensor(out=prod[:, :], in0=covf[:, :],
                                        in1=tbct[:, :], op=OP.mult)
                pool_tree_reduce(prod, vT_sb, blk * NGRP, NGRP)

            def pool_tree_reduce(buf, dst, col0, ngrp):
                # in-place halving adds on Pool; final add lands in dst
                w = 64
                while w >= 1:
                    src = buf[:, :].rearrange("b (g j) -> b g j", j=128)
                    in0 = src[:, :, 0:w]
                    in1 = src[:, :, w:2 * w]
                    if w == 1:
                        out = dst[:, col0:col0 + ngrp].rearrange(
                            "b (g o) -> b g o", o=1)
                    else:
                        out = in0
                    nc.gpsimd.tensor_tensor(out=out, in0=in0, in1=in1,
                                            op=OP.add)
                    w //= 2

            # ---------- phase 2 helper (interleaved into block 7) ----
            def phase2(c):
                lhsT = xt_sb[:, c * 128:(c + 1) * 128]
                t1a = t1a_all[c]
                pl = psp.tile([128, K], FP32, tag="ps")
                nc.tensor.matmul(pl[:, :], lhsT, vT_sb[:, :],
                                 start=True, stop=False)
                nc.tensor.matmul(pl[:, :], ones_row[:, :], const_row[:, :],
                                 start=False, stop=True)
                lg = workp.tile([128, K], FP32, tag="lg")
                nc.vector.tensor_tensor(out=lg[:, :], in0=pl[:, :],
                                        in1=t1a[:, :], op=OP.subtract)
                mx = workp.tile([128, 1], FP32, tag="mx")
                nc.vector.tensor_reduce(out=mx[:, :], in_=lg[:, :],
                                        axis=AX.X, op=OP.max)
                nmx = workp.tile([128, 1], FP32, tag="nmx")
                nc.vector.tensor_scalar_mul(nmx[:, :], mx[:, :], -1.0)
                ex = workp.tile([128, K], FP32, tag="ex")
                den = workp.tile([128, 1], FP32, tag="den")
                nc.scalar.activation(ex[:, :], lg[:, :], AF.Exp,
                                     bias=nmx[:, 0:1], accum_out=den[:, 0:1])
                rden = workp.tile([128, 1], FP32, tag="rden")
                nc.vector.reciprocal(rden[:, :], den[:, :])
                ot = workp.tile([128, K], FP32, tag="ot")
                nc.vector.tensor_scalar(out=ot[:, :], in0=ex[:, :],
                                        scalar1=rden[:, 0:1], scalar2=None,
                                        op0=OP.mult)
                nc.sync.dma_start(out=out_d[c * 128:(c + 1) * 128, :],
                                  in_=ot[:, :])

            # ---------- phase 1: blocks outer, chunks inner ----------
            # t/v stages for block blk+1 are emitted at chunk boundaries
            # c4..c7 of block blk (block 0 runs its own at c0..c3), so v
            # completes one block ahead and phase 2 can interleave into
            # block 7.
            cur = (covf0, chrt0, ecbt0, cbbt0)
            nxt = None
            covf_nxt = None
            tbc = {}
            tkj = {}
            for blk in range(N_BLK):
                covf_cur, chrt, ecbt, cbbt = cur
                if blk + 1 < N_BLK:
                    covf_nxt = load_block(blk + 1)
                    nxt = (covf_nxt,) + conv_block(covf_nxt, blk + 1)

                for c in range(N_CHUNKS):
                    t1a = t1a_all[c]
                    for h in range(2):
                        py = pyp.tile([128, HB], FP32, tag="py")
                        o = h * HB
                        for m in range(HB // 512):
                            nc.tensor.matmul(
                                py[:, m * 512:(m + 1) * 512],
                                xtr[:, c * 128:(c + 1) * 128],
                                chrt[:, o + m * 512:o + (m + 1) * 512],
                                start=True, stop=False,
                                skip_group_check=True)
                        for m in range(HB // 512):
                            nc.tensor.matmul(
                                py[:, m * 512:(m + 1) * 512],
                                exb[:, c * 128:(c + 1) * 128],
                                cbbt[:, o + m * 512:o + (m + 1) * 512],
                                start=False, stop=False,
                                skip_group_check=True)
                        for m in range(HB // 512):
                            nc.tensor.matmul(
                                py[:, m * 512:(m + 1) * 512],
                                xbb[:, c * 128:(c + 1) * 128],
                                ecbt[:, o + m * 512:o + (m + 1) * 512],
                                start=False, stop=True,
                                skip_group_check=True)

                        ysq = ysqp.tile([128, HB], FP32, tag="ysq")
                        nc.scalar.activation(ysq[:, :], py[:, :], AF.Square,
                                             scale=SQRT_A)
                        col0 = blk * NGRP + h * HGRP
                        nc.vector.tensor_reduce(
                            out=t1a[:, col0:col0 + HGRP],
                            in_=ysq[:, :].rearrange(
                                "b (g j) -> b g j", j=128),
                            axis=AX.X, op=OP.add)

                    # t/v pipeline stages at chunk boundaries (ready-order
                    # per engine queue)
                    if blk == 0:
                        if c == 0:
                            t_stage1(0, covf_cur)
                        elif c == 1:
                            tkj[0] = t_stage2(0)
                        elif c == 2:
                            tbc[0] = t_stage3(0, tkj[0])
                        elif c == 3:
                            t_stage4(0, covf_cur, tbc[0])
                    if blk + 1 < N_BLK:
                        if c == 4:
                            t_stage1(blk + 1, covf_nxt)
                        elif c == 5:
                            tkj[blk + 1] = t_stage2(blk + 1)
                        elif c == 6:
                            tbc[blk + 1] = t_stage3(blk + 1, tkj[blk + 1])
                        elif c == 7:
                            t_stage4(blk + 1, covf_nxt, tbc[blk + 1])
                    if blk == N_BLK - 1:
                        if c == 0:
                            const_chain()
                        if c >= 2:
                            phase2(c - 2)
                cur = nxt

            phase2(6)
            phase2(7)

    nc.compile()
    return nc


_NC_CACHE = None


def kernel(x, centers, cov, mixers):
    global _NC_CACHE
    from concourse.bass_utils import run_bass_kernel_spmd

    if _NC_CACHE is None:
        _NC_CACHE = _build_bass()
    nc = _NC_CACHE

    x = np.ascontiguousarray(x, dtype=np.float32)
    cov2 = np.ascontiguousarray(cov, dtype=np.float32).reshape(K * D, D)
    cen = np.ascontiguousarray(centers, dtype=np.float32)
    mix = np.ascontiguousarray(mixers, dtype=np.float32)
    ident = np.eye(128, dtype=np.float32)

    in_maps = []
    for c in range(N_CORES):
        in_maps.append({
            "x": x[c * B_LOC:(c + 1) * B_LOC],
            "cov": cov2,
            "centers": cen,
            "mixers": mix,
            "ident": ident,
        })
    res = run_bass_kernel_spmd(nc, in_maps, list(range(N_CORES)))
    out = np.concatenate([res.results[c]["out"] for c in range(N_CORES)],
                         axis=0)
    return out
